# revision 10
# baseline (speedup 1.0000x reference)
import sys
sys.path.insert(0, '/opt/trn_rl_repo')
import time as _time
from concurrent.futures import ThreadPoolExecutor
import numpy as np
import concourse.bass as bass
import concourse.mybir as mybir
import concourse.tile as tile
import concourse.bacc as bacc
from concourse.bass_utils import run_bass_kernel_spmd
import ml_dtypes

NC = 8
P = 128
NGRP = 8           # gpsimd groups (16 partitions each)
NUM_GRAPHS = 256
GPC = NUM_GRAPHS // NC
N_NODES = 200_000

TRACE = False
LAST_EXEC_NS = []
LAST_RESULTS = []
LAST_WALL_S = []
_PROG_CACHE = {}

f16 = mybir.dt.float16
f8 = mybir.dt.float8e4
F8 = ml_dtypes.float8_e4m3
f32 = mybir.dt.float32
u8 = mybir.dt.uint8
i16 = mybir.dt.int16
ADD = mybir.AluOpType.add
MULT = mybir.AluOpType.mult
ISEQ = mybir.AluOpType.is_equal
ISGT = mybir.AluOpType.is_gt
SHL = mybir.AluOpType.logical_shift_left
SHR = mybir.AluOpType.logical_shift_right
BAND = mybir.AluOpType.bitwise_and
BOR = mybir.AluOpType.bitwise_or


def _dp_bins(hists, maxw, gran, slotmul, pen):
    """Width-binning DP. hists [nlists, maxw+1]: per-list counts per width.
    Bin (lo,hi] holds rows=ceil(maxcnt/gran) rows of hi; cost rows*slotmul*hi.
    Returns [(w, rows, col0)], COLS (=sum rows*w), NROWS."""
    Ccum = np.zeros((hists.shape[0], maxw + 1), np.int64)
    Ccum[:, 1:] = np.cumsum(hists[:, 1:], axis=1)
    INF = float('inf')
    dp = np.full(maxw + 1, INF)
    dp[0] = 0.0
    choice = np.zeros(maxw + 1, np.int64)
    for j in range(1, maxw + 1):
        for i in range(j):
            cnt = int((Ccum[:, j] - Ccum[:, i]).max())
            if cnt == 0:
                cost = dp[i]
            else:
                rows = -(-cnt // gran)
                cost = dp[i] + rows * slotmul * j + pen
            if cost < dp[j]:
                dp[j] = cost
                choice[j] = i
    bnds = []
    j = maxw
    while j > 0:
        i = int(choice[j])
        bnds.append((i, j))
        j = i
    bnds.reverse()
    bins = []
    col = 0
    nrows = 0
    for (i, j) in bnds:
        cnt = int((Ccum[:, j] - Ccum[:, i]).max())
        if cnt == 0:
            continue
        rows = -(-cnt // gran)
        bins.append((j, rows, col))
        col += rows * j
        nrows += rows
    return bins, col, nrows


# ---------------------------------------------------------------- staging

def _stage_l1(n0, V, src_g, dst_l, Wn, bins, COLS, NR, z1, dinv):
    order = np.argsort(Wn, kind='stable')
    Wo = Wn[order]
    es = np.argsort(dst_l, kind='stable')
    src_sorted = src_g[es]
    row_ptr = np.searchsorted(dst_l[es], np.arange(V + 1))
    slot_src = np.full(P * COLS, N_NODES, np.int64)
    dinvn = np.zeros(P * NR, np.float16)
    node_map = np.full(P * NR, -1, np.int64)
    ptr = 0
    nodecol = 0
    for (w, rows, col0) in bins:
        nb = int(np.searchsorted(Wo, w, side='right')) - ptr
        nodes = order[ptr:ptr + nb]
        Wb = Wo[ptr:ptr + nb]
        ptr += nb
        if nb:
            i = np.arange(nb)
            p = i % P
            r = i // P
            flat = p * COLS + col0 + r * w
            cnts = Wb - 1
            tot = int(cnts.sum())
            if tot:
                c0 = np.zeros(nb, np.int64)
                np.cumsum(cnts[:-1], out=c0[1:])
                rep = np.repeat(np.arange(nb), cnts)
                intra = np.arange(tot) - c0[rep]
                slot_src[flat[rep] + intra] = src_sorted[row_ptr[nodes][rep] + intra]
            slot_src[flat + cnts] = n0 + nodes
            gi = p * NR + nodecol + r
            dinvn[gi] = dinv[n0 + nodes].astype(np.float16)
            node_map[gi] = nodes
        nodecol += rows
    xp = z1[slot_src].view(F8)  # [P*COLS, 4] fp8
    return xp, dinvn.reshape(P, NR), node_map.reshape(P, NR)


def _stage_l2_blk(n0, V, src_l, dst_g, bins2, COLS2, NRg,
                  deg_u8_g, batch_u8_g, l1pos):
    """Block-layout src-sharded L2: node -> (group g, row r); its padded
    16*w16 slots fill partitions [16g,16g+16) x cols [col0+r*w16, +w16).

    Returns gid u8 [P*COLS2], degd u8 [P*COLS2] (pad 0 -> masked),
    pidx int16 [128, NRI//16] ap_gather indices, NRI (padded row count).
    """
    w2 = np.bincount(src_l, minlength=V)  # real out-edges
    W16 = (w2 + 1 + 15) // 16             # slots incl self, /16 blocks
    es = np.argsort(src_l, kind='stable')
    dst_sorted = dst_g[es]
    eptr = np.searchsorted(src_l[es], np.arange(V + 1))
    order = np.argsort(W16, kind='stable')
    Wo = W16[order]
    gid = np.zeros(P * COLS2, np.uint8)
    degd = np.zeros(P * COLS2, np.uint8)
    NRI = -(-NRg // 16) * 16
    Lg = np.zeros((NGRP, NRI), np.int64)
    ptr = 0
    rowbase = 0
    for (w16, rowsb, col0) in bins2:
        nb = int(np.searchsorted(Wo, w16, side='right')) - ptr
        nodes = order[ptr:ptr + nb]
        ptr += nb
        if nb:
            k = np.arange(nb)
            g = k % NGRP
            r = rowbase + k // NGRP
            Lg[g, r] = l1pos[nodes]
            cnts = w2[nodes]
            tot = int(cnts.sum())
            colb = col0 + (k // NGRP) * w16
            if tot:
                c0 = np.zeros(nb, np.int64)
                np.cumsum(cnts[:-1], out=c0[1:])
                rep = np.repeat(np.arange(nb), cnts)
                intra = np.arange(tot) - c0[rep]
                ds = dst_sorted[eptr[nodes][rep] + intra]
                cpart = intra % 16
                jj = intra // 16
                pos = (16 * g[rep] + cpart) * COLS2 + colb[rep] + jj
                gid[pos] = batch_u8_g[ds]
                degd[pos] = deg_u8_g[ds]
            cs = cnts % 16
            js = cnts // 16
            pos_s = (16 * g + cs) * COLS2 + colb + js
            gid[pos_s] = batch_u8_g[n0 + nodes]
            degd[pos_s] = deg_u8_g[n0 + nodes]
        rowbase += rowsb
    pidx = np.zeros((128, NRI // 16), np.int16)
    for g in range(NGRP):
        pidx[16 * g:16 * (g + 1), :] = Lg[g].reshape(NRI // 16, 16).T
    return gid, degd, pidx, NRI


# ---------------------------------------------------------------- program

def _bcast_last(ap, k):
    return bass.AP(ap.tensor, ap.offset, list(ap.ap) + [[0, k]])


def _zero_nr(ap, n):
    return bass.AP(ap.tensor, ap.offset, [ap.ap[0], [0, n], ap.ap[1]])


def _build_fused_program(bins, COLS, NR, bins2, COLS2, NRI, dc0, dc1):
    NE = P * NR  # m-table rows
    assert NE <= 2 ** 15, "m table exceeds int16/ap_gather range"
    nc = bacc.Bacc("TRN2", target_bir_lowering=False, debug=False,
                   num_devices=NC)
    x_in = nc.dram_tensor("xp", [P * COLS, 4], f8, kind="ExternalInput")
    a_dv = 448
    a_pi = a_dv + 2 * NR
    MB = -(-(a_pi + 2 * (NRI // 16)) // 4) * 4
    misc_in = nc.dram_tensor("misc", [P, MB], u8, kind="ExternalInput")
    COL2E = COLS2 + (COLS2 & 1)
    gd_in = nc.dram_tensor("gd", [P, COLS2 + COL2E // 2], u8,
                           kind="ExternalInput")
    out = nc.dram_tensor("out", [1, 2 * NUM_GRAPHS], f32,
                         kind="ExternalOutput")

    with tile.TileContext(nc) as tc:
        with (
            tc.tile_pool(name="keep", bufs=1) as kp,
            tc.tile_pool(name="dram", bufs=1, space="DRAM") as dp,
            tc.tile_pool(name="ps", bufs=1, space="PSUM") as ps,
        ):
            gath = kp.tile([P, NRI, 2], f16, tag="gath")

            # ---------------- phase A: conv1 -> m (local nodes, L1 grid)
            with tc.tile_pool(name="ph_a", bufs=1) as ap_:
                xs = ap_.tile([P, COLS, 4], f8, tag="xs")
                nc.sync.dma_start(xs[:], x_in.ap().rearrange("(p c) d -> p c d", p=P))
                dv16 = ap_.tile([P, NR], f16, tag="dv16")
                nc.sync.dma_start(dv16[:],
                                  misc_in.ap()[:, a_dv:a_dv + 2 * NR].bitcast(f16))
                w1 = ap_.tile([P, 64], f32, tag="w1")
                nc.sync.dma_start(w1[:], misc_in.ap()[:, 0:256].bitcast(f32))
                b1 = ap_.tile([P, 16], f32, tag="b1")
                nc.sync.dma_start(b1[:], misc_in.ap()[:, 256:320].bitcast(f32))
                w2 = ap_.tile([P, 32], f32, tag="w2")
                nc.sync.dma_start(w2[:], misc_in.ap()[:, 320:448].bitcast(f32))

                a1 = ap_.tile([P, NR, 4], f32, tag="a1")
                nodecol = 0
                for (w, rows, col0) in bins:
                    sl = xs[:, col0:col0 + rows * w, :]
                    cs = sl.ap[1][0]
                    slT = bass.AP(sl.tensor, sl.offset,
                                  [sl.ap[0], [cs * w, rows], sl.ap[2], [cs, w]])
                    nc.vector.tensor_reduce(
                        out=a1[:, nodecol:nodecol + rows, :], in_=slT,
                        axis=mybir.AxisListType.X, op=ADD)
                    nodecol += rows

                dv = ap_.tile([P, NR], f32, tag="dv")
                nc.vector.tensor_copy(dv[:], dv16[:])
                a1p = ap_.tile([P, NR, 4], f32, tag="a1p")
                nc.vector.tensor_tensor(out=a1p[:], in0=a1[:],
                                        in1=_bcast_last(dv[:], 4), op=MULT)

                h = ap_.tile([P, NR, 16], f32, tag="h")
                tmp = ap_.tile([P, NR, 16], f32, tag="tmp")
                for f in range(4):
                    af = a1p[:, :, f:f + 1]
                    afb = bass.AP(af.tensor, af.offset,
                                  [af.ap[0], af.ap[1], [0, 16]])
                    wfb = _zero_nr(w1[:, f * 16:(f + 1) * 16], NR)
                    if f == 0:
                        nc.vector.tensor_tensor(out=h[:], in0=afb, in1=wfb, op=MULT)
                    else:
                        nc.vector.tensor_tensor(out=tmp[:], in0=afb, in1=wfb, op=MULT)
                        nc.vector.tensor_tensor(out=h[:], in0=h[:], in1=tmp[:], op=ADD)
                nc.vector.tensor_tensor(out=h[:], in0=h[:],
                                        in1=_zero_nr(b1[:], NR), op=ADD)
                nc.vector.tensor_scalar_max(h[:], h[:], 0.0)

                mt = ap_.tile([P, NR, 2], f32, tag="mt")
                for o in range(2):
                    wob = _zero_nr(w2[:, o * 16:(o + 1) * 16], NR)
                    nc.vector.tensor_tensor(out=tmp[:], in0=h[:], in1=wob, op=MULT)
                    nc.vector.tensor_reduce(out=mt[:, :, o], in_=tmp[:],
                                            axis=mybir.AxisListType.X, op=ADD)
                mo = ap_.tile([P, NR, 2], f16, tag="mo")
                nc.vector.tensor_tensor(out=mo[:], in0=mt[:],
                                        in1=_bcast_last(dv[:], 2), op=MULT)
                scratch = dp.tile([P * NR, 2], f16, tag="scratch")
                nc.sync.dma_start(
                    scratch[:].rearrange("(p c) d -> p c d", p=P), mo[:])

            # ---------------- phase B: permute m to L2 block grid
            with tc.tile_pool(name="ph_b", bufs=1) as bp:
                table = bp.tile([P, NE, 2], f16, tag="table")
                sc = scratch[:]
                bcast = bass.AP(sc.tensor, sc.offset, [[0, P], [2, NE], [1, 2]])
                nc.sync.dma_start(table[:], bcast)
                pidx = bp.tile([P, NRI // 16], i16, tag="pidx")
                nc.sync.dma_start(
                    pidx[:],
                    misc_in.ap()[:, a_pi:a_pi + 2 * (NRI // 16)].bitcast(i16))
                nc.gpsimd.ap_gather(gath[:], table[:], pidx[:],
                                    channels=P, num_elems=NE, d=2,
                                    num_idxs=NRI)

            # ---------------- phase C: conv2 + pooling over all graphs
            with tc.tile_pool(name="ph_c", bufs=1) as cpl:
                gt = cpl.tile([P, COLS2], u8, tag="gt")
                nc.sync.dma_start(gt[:], gd_in.ap()[:, :COLS2])
                dp4 = cpl.tile([P, COL2E // 2], u8, tag="dp4")
                nc.sync.dma_start(dp4[:], gd_in.ap()[:, COLS2:])
                ddc = cpl.tile([P, COL2E], u8, tag="ddc")
                dv_ = ddc[:].rearrange("p (c two) -> p c two", two=2)
                nc.vector.tensor_scalar(out=dv_[:, :, 0], in0=dp4[:],
                                        scalar1=15, scalar2=None, op0=BAND)
                nc.vector.tensor_scalar(out=dv_[:, :, 1], in0=dp4[:],
                                        scalar1=4, scalar2=15,
                                        op0=SHR, op1=BAND)
                din = cpl.tile([P, COLS2], f32, tag="din")
                junk = cpl.tile([P, COLS2], f32, tag="junk")
                nc.vector.tensor_scalar(out=junk[:], in0=ddc[:, :COLS2],
                                        scalar1=0.5, scalar2=None, op0=ISGT)
                dcb = cpl.tile([P, 1], f32, tag="dcb")
                nc.vector.memset(dcb[:], float(dc0))
                nc.scalar.activation(din[:], ddc[:, :COLS2],
                                     mybir.ActivationFunctionType.Exp,
                                     bias=dcb[:, 0:1], scale=float(dc1))
                nc.vector.tensor_tensor(out=din[:], in0=din[:], in1=junk[:],
                                        op=MULT)

                v = cpl.tile([P, COLS2, 2], f16, tag="v")
                rowbase = 0
                for (w16, rowsb, col0) in bins2:
                    dst = v[:, col0:col0 + rowsb * w16, :]
                    cs = dst.ap[1][0]
                    dstv = bass.AP(dst.tensor, dst.offset,
                                   [dst.ap[0], [cs * w16, rowsb], [cs, w16],
                                    dst.ap[2]])
                    src = gath[:, rowbase:rowbase + rowsb, :]
                    srcb = bass.AP(src.tensor, src.offset,
                                   [src.ap[0], src.ap[1], [0, w16], src.ap[2]])
                    nc.vector.tensor_copy(dstv, srcb)
                    rowbase += rowsb
                nc.vector.tensor_tensor(out=v[:], in0=v[:],
                                        in1=_bcast_last(din[:], 2), op=MULT)

                po = cpl.tile([P, 2 * NUM_GRAPHS], f32, tag="po")
                for g in range(NUM_GRAPHS):
                    for o in range(2):
                        nc.vector.scalar_tensor_tensor(
                            out=junk[:], in0=gt[:], scalar=float(g),
                            in1=v[:, :, o], op0=ISEQ, op1=MULT,
                            accum_out=po[:, 2 * g + o:2 * g + o + 1])

                ones = cpl.tile([P, 1], f32, tag="ones")
                nc.vector.memset(ones[:], 1.0)
                acc = ps.tile([1, 2 * NUM_GRAPHS], f32, tag="acc")
                nc.tensor.matmul(acc[:], lhsT=ones[:], rhs=po[:],
                                 start=True, stop=True)
                ot = cpl.tile([1, 2 * NUM_GRAPHS], f32, tag="ot")
                nc.vector.tensor_copy(ot[:], acc[:])
                nc.sync.dma_start(out.ap(), ot[:])
    nc.compile()
    return nc


# ---------------------------------------------------------------- pipeline
# Cache the jitted PJRT executable per program: bass2jax.run_bass_via_pjrt
# rebuilds its jit closure every call, paying a re-trace each launch. The
# cached path runs the identical computation (same custom call, transfers,
# donation); any failure falls back to the original implementation.
import concourse.bass2jax as _b2j

_ORIG_RVP = _b2j.run_bass_via_pjrt
_EXE_CACHE = {}


def _cached_rvp(nc, in_maps, n_cores):
    try:
        import jax
        from jax.sharding import Mesh, PartitionSpec
        from jax.experimental.shard_map import shard_map
        key = (id(nc), n_cores)
        ent = _EXE_CACHE.get(key)
        if ent is None:
            _b2j.install_neuronx_cc_hook()
            if nc.dbg_addr is not None or nc.partition_id_tensor is not None:
                return _ORIG_RVP(nc, in_maps, n_cores)
            in_names, out_names, out_avals = [], [], []
            for alloc in nc.m.functions[0].allocations:
                if not isinstance(alloc, mybir.MemoryLocationSet):
                    continue
                name = alloc.memorylocations[0].name
                if alloc.kind == "ExternalInput":
                    in_names.append(name)
                elif alloc.kind == "ExternalOutput":
                    shape = tuple(alloc.tensor_shape)
                    dtype = mybir.dt.np(alloc.dtype)
                    out_names.append(name)
                    out_avals.append(jax.core.ShapedArray(shape, dtype))
            n_params = len(in_names)
            all_names = in_names + out_names
            donate = tuple(range(n_params, n_params + len(out_avals)))

            def _body(*args):
                return tuple(_b2j._bass_exec_p.bind(
                    *args, out_avals=tuple(out_avals),
                    in_names=tuple(all_names), out_names=tuple(out_names),
                    lowering_input_output_aliases=(),
                    sim_require_finite=True, sim_require_nnan=True, nc=nc))

            devices = jax.devices()[:n_cores]
            mesh = Mesh(np.asarray(devices), ("core",))
            nio = n_params + len(out_avals)
            sharded = jax.jit(
                shard_map(_body, mesh=mesh,
                          in_specs=(PartitionSpec("core"),) * nio,
                          out_specs=(PartitionSpec("core"),) * len(out_names),
                          check_rep=False),
                donate_argnums=donate, keep_unused=True)
            ent = (sharded, in_names, n_params, out_names, out_avals)
            _EXE_CACHE[key] = ent
        sharded, in_names, n_params, out_names, out_avals = ent
        per_core = [[np.asarray(m[name]) for name in in_names[:n_params]]
                    for m in in_maps]
        concat_in = [np.concatenate([per_core[c][i] for c in range(n_cores)],
                                    axis=0) for i in range(n_params)]
        concat_zeros = [np.zeros((n_cores * a.shape[0], *a.shape[1:]), a.dtype)
                        for a in out_avals]
        out_arrs = sharded(*concat_in, *concat_zeros)
        return [
            {name: np.asarray(out_arrs[i]).reshape(
                n_cores, *out_avals[i].shape)[c]
             for i, name in enumerate(out_names)}
            for c in range(n_cores)
        ]
    except Exception:
        _EXE_CACHE.pop((id(nc), n_cores), None)
        return _ORIG_RVP(nc, in_maps, n_cores)


_b2j.run_bass_via_pjrt = _cached_rvp


def _hw_runner(nc, in_maps):
    try:
        return run_bass_kernel_spmd(nc, in_maps, core_ids=list(range(NC)),
                                    trace=TRACE)
    except ModuleNotFoundError:
        return run_bass_kernel_spmd(nc, in_maps, core_ids=list(range(NC)))


def kernel(x, edge_index, batch, W1, b1, W2, b2, Wl, bl, _runner=None):
    runner = _runner or _hw_runner
    x = np.asarray(x, np.float32)
    edge_index = np.asarray(edge_index)
    batch = np.asarray(batch).astype(np.int64)
    N = x.shape[0]
    assert N == N_NODES
    src = edge_index[0].astype(np.int64)
    dst = edge_index[1].astype(np.int64)

    deg_i = np.bincount(dst, minlength=N) + 1
    dinv = 1.0 / np.sqrt(deg_i.astype(np.float32))
    z1f = np.zeros((N + 1, 4), np.float32)
    z1f[:N] = x * dinv[:, None]
    z1b = z1f.astype(F8).view(np.uint8)  # [N+1, 4] fp8 bytes
    DLO = float(deg_i.min())
    DHI = float(max(deg_i.max(), deg_i.min() + 1))
    NLV = 15
    dstep = np.log(DHI / DLO) / (NLV - 1)
    kq = np.clip(np.round(np.log(deg_i / DLO) / dstep), 0, NLV - 1)
    degc_u8_g = (kq + 1).astype(np.uint8)  # 0 reserved for pad
    dc1 = -0.5 * dstep
    dc0 = -0.5 * np.log(DLO) + 0.5 * dstep
    batch_u8_g = batch.astype(np.uint8)

    gb = np.searchsorted(batch, np.arange(NUM_GRAPHS + 1))
    nbounds = [int(gb[GPC * c]) for c in range(NC)] + [N]

    ex = ThreadPoolExecutor(2 * NC)

    def _extract1(c):
        n0, n1 = nbounds[c], nbounds[c + 1]
        mask = (dst >= n0) & (dst < n1)
        return (n0, n1 - n0, src[mask], dst[mask] - n0)

    def _extract2(c):
        n0, n1 = nbounds[c], nbounds[c + 1]
        mask = (src >= n0) & (src < n1)
        sl = src[mask] - n0
        dg = dst[mask]
        w2 = np.bincount(sl, minlength=n1 - n0)
        return sl, dg, (w2 + 1 + 15) // 16

    fut1 = [ex.submit(_extract1, c) for c in range(NC)]
    fut2 = [ex.submit(_extract2, c) for c in range(NC)]
    cores = [f.result() for f in fut1]
    l2ex = [f.result() for f in fut2]
    cores2 = [(sl, dg) for (sl, dg, _) in l2ex]

    maxw = int(deg_i.max())
    hh1 = np.zeros((NC, maxw + 1), np.int64)
    for c, (n0, V, s, d) in enumerate(cores):
        hh1[c] = np.bincount(deg_i[n0:n0 + V], minlength=maxw + 1)
    bins, COLS, NR = _dp_bins(hh1, maxw, P, P, 2048)

    w16max = max(int(W16.max()) for (_, _, W16) in l2ex)
    hh2 = np.zeros((NC * NGRP, w16max + 1), np.int64)
    for c in range(NC):
        # deal within sorted order -> per-group counts differ by <=1;
        # conservatively use ceil(count/NGRP) per width as per-group count
        hc = np.bincount(l2ex[c][2], minlength=w16max + 1)
        hh2[c * NGRP] = -(-hc // NGRP)
    bins2, COLS2, NRg = _dp_bins(hh2, w16max, 1, 16, 256)

    def _stage_core(c):
        n0, n1 = nbounds[c], nbounds[c + 1]
        V = n1 - n0
        st1 = _stage_l1(n0, V, cores[c][2], cores[c][3], deg_i[n0:n0 + V],
                        bins, COLS, NR, z1b, dinv)
        node_map = st1[2]
        l1pos = np.zeros(V, np.int64)
        nm = node_map.reshape(-1)
        valid = nm >= 0
        l1pos[nm[valid]] = np.arange(P * NR)[valid]
        sl, dg = cores2[c]
        st2 = _stage_l2_blk(n0, V, sl, dg, bins2, COLS2, NRg,
                            degc_u8_g, batch_u8_g, l1pos)
        return st1, st2

    staged = list(ex.map(_stage_core, range(NC)))
    ex.shutdown(wait=False)
    l1_stage = [s[0] for s in staged]
    l2_stage = [s[1] for s in staged]
    NRI = l2_stage[0][3]

    W1 = np.asarray(W1, np.float32)
    W2p = np.asarray(W2, np.float32) @ np.asarray(Wl, np.float32)
    W1r = np.broadcast_to(W1.reshape(1, 64), (P, 64)).copy()
    b1r = np.broadcast_to(np.asarray(b1, np.float32).reshape(1, 16),
                          (P, 16)).copy()
    W2pr = np.broadcast_to(W2p.T.reshape(1, 32), (P, 32)).copy()

    key = ("fused", tuple(bins), tuple(bins2),
           round(float(dc0), 9), round(float(dc1), 9))
    if key not in _PROG_CACHE:
        _PROG_CACHE[key] = _build_fused_program(bins, COLS, NR,
                                                bins2, COLS2, NRI, dc0, dc1)
    nc = _PROG_CACHE[key]
    a_dv = 448
    a_pi = a_dv + 2 * NR
    MB = -(-(a_pi + 2 * (NRI // 16)) // 4) * 4
    in_maps = []
    for c in range(NC):
        x_slot, dinvn, _ = l1_stage[c]
        gid, degd, pidx, _ = l2_stage[c]
        misc = np.zeros((P, MB), np.uint8)
        misc[:, 0:256] = W1r.view(np.uint8)
        misc[:, 256:320] = b1r.view(np.uint8)
        misc[:, 320:448] = W2pr.view(np.uint8)
        misc[:, a_dv:a_dv + 2 * NR] = dinvn.view(np.uint8)
        misc[:, a_pi:a_pi + 2 * (NRI // 16)] = pidx.view(np.uint8)
        COL2E = COLS2 + (COLS2 & 1)
        dge = np.zeros((P, COL2E), np.uint8)
        dge[:, :COLS2] = degd.reshape(P, COLS2)
        dp4 = (dge[:, 0::2] | (dge[:, 1::2] << 4)).astype(np.uint8)
        gd = np.concatenate([gid.reshape(P, COLS2), dp4], axis=1)
        in_maps.append({"xp": x_slot, "misc": misc, "gd": gd})
    t0 = _time.time()
    res = runner(nc, in_maps)
    LAST_WALL_S.append(_time.time() - t0)
    LAST_RESULTS.append(res)
    LAST_EXEC_NS.append(res.exec_time_ns)

    bias = (np.asarray(b2, np.float32) @ np.asarray(Wl, np.float32)
            + np.asarray(bl, np.float32))
    sizes = np.diff(gb).astype(np.float32)
    acc = np.zeros((NUM_GRAPHS, 2), np.float32)
    for c in range(NC):
        acc += np.asarray(res.results[c]["out"]).reshape(NUM_GRAPHS, 2)
    out = acc / np.maximum(sizes, 1.0)[:, None] + bias[None, :]
    out[sizes == 0] = 0.0
    return out


# revision 11
# speedup vs baseline: 1.7349x; 1.7349x over previous
import sys
sys.path.insert(0, '/opt/trn_rl_repo')
import time as _time
from concurrent.futures import ThreadPoolExecutor
import numpy as np
import concourse.bass as bass
import concourse.mybir as mybir
import concourse.tile as tile
import concourse.bacc as bacc
from concourse.bass_utils import run_bass_kernel_spmd
import ml_dtypes

NC = 8
P = 128
NGRP = 8           # gpsimd groups (16 partitions each)
NUM_GRAPHS = 256
GPC = NUM_GRAPHS // NC
N_NODES = 200_000

TRACE = False
LAST_EXEC_NS = []
LAST_RESULTS = []
LAST_WALL_S = []
_PROG_CACHE = {}

f16 = mybir.dt.float16
f8 = mybir.dt.float8e4
F8 = ml_dtypes.float8_e4m3
f32 = mybir.dt.float32
u8 = mybir.dt.uint8
i16 = mybir.dt.int16
ADD = mybir.AluOpType.add
MULT = mybir.AluOpType.mult
ISEQ = mybir.AluOpType.is_equal
ISGT = mybir.AluOpType.is_gt
SHL = mybir.AluOpType.logical_shift_left
SHR = mybir.AluOpType.logical_shift_right
BAND = mybir.AluOpType.bitwise_and
BOR = mybir.AluOpType.bitwise_or


def _dp_bins(hists, maxw, gran, slotmul, pen):
    """Width-binning DP. hists [nlists, maxw+1]: per-list counts per width.
    Bin (lo,hi] holds rows=ceil(maxcnt/gran) rows of hi; cost rows*slotmul*hi.
    Returns [(w, rows, col0)], COLS (=sum rows*w), NROWS."""
    Ccum = np.zeros((hists.shape[0], maxw + 1), np.int64)
    Ccum[:, 1:] = np.cumsum(hists[:, 1:], axis=1)
    INF = float('inf')
    dp = np.full(maxw + 1, INF)
    dp[0] = 0.0
    choice = np.zeros(maxw + 1, np.int64)
    for j in range(1, maxw + 1):
        for i in range(j):
            cnt = int((Ccum[:, j] - Ccum[:, i]).max())
            if cnt == 0:
                cost = dp[i]
            else:
                rows = -(-cnt // gran)
                cost = dp[i] + rows * slotmul * j + pen
            if cost < dp[j]:
                dp[j] = cost
                choice[j] = i
    bnds = []
    j = maxw
    while j > 0:
        i = int(choice[j])
        bnds.append((i, j))
        j = i
    bnds.reverse()
    bins = []
    col = 0
    nrows = 0
    for (i, j) in bnds:
        cnt = int((Ccum[:, j] - Ccum[:, i]).max())
        if cnt == 0:
            continue
        rows = -(-cnt // gran)
        bins.append((j, rows, col))
        col += rows * j
        nrows += rows
    return bins, col, nrows


# ---------------------------------------------------------------- staging

def _stage_l1(n0, V, src_g, dst_l, Wn, bins, COLS, NR, z1, dinv):
    order = np.argsort(Wn, kind='stable')
    Wo = Wn[order]
    es = np.argsort(dst_l, kind='stable')
    src_sorted = src_g[es]
    row_ptr = np.searchsorted(dst_l[es], np.arange(V + 1))
    slot_src = np.full(P * COLS, N_NODES, np.int64)
    dinvn = np.zeros(P * NR, np.float16)
    node_map = np.full(P * NR, -1, np.int64)
    ptr = 0
    nodecol = 0
    for (w, rows, col0) in bins:
        nb = int(np.searchsorted(Wo, w, side='right')) - ptr
        nodes = order[ptr:ptr + nb]
        Wb = Wo[ptr:ptr + nb]
        ptr += nb
        if nb:
            i = np.arange(nb)
            p = i % P
            r = i // P
            flat = p * COLS + col0 + r * w
            cnts = Wb - 1
            tot = int(cnts.sum())
            if tot:
                c0 = np.zeros(nb, np.int64)
                np.cumsum(cnts[:-1], out=c0[1:])
                rep = np.repeat(np.arange(nb), cnts)
                intra = np.arange(tot) - c0[rep]
                slot_src[flat[rep] + intra] = src_sorted[row_ptr[nodes][rep] + intra]
            slot_src[flat + cnts] = n0 + nodes
            gi = p * NR + nodecol + r
            dinvn[gi] = dinv[n0 + nodes].astype(np.float16)
            node_map[gi] = nodes
        nodecol += rows
    xp = z1[slot_src].view(F8)  # [P*COLS, 4] fp8
    return xp, dinvn.reshape(P, NR), node_map.reshape(P, NR)


def _stage_l2_blk(n0, V, src_l, dst_g, bins2, COLS2, NRg,
                  deg_u8_g, batch_u8_g, l1pos):
    """Block-layout src-sharded L2: node -> (group g, row r); its padded
    16*w16 slots fill partitions [16g,16g+16) x cols [col0+r*w16, +w16).

    Returns gid u8 [P*COLS2], degd u8 [P*COLS2] (pad 0 -> masked),
    pidx int16 [128, NRI//16] ap_gather indices, NRI (padded row count).
    """
    w2 = np.bincount(src_l, minlength=V)  # real out-edges
    W16 = (w2 + 1 + 15) // 16             # slots incl self, /16 blocks
    es = np.argsort(src_l, kind='stable')
    dst_sorted = dst_g[es]
    eptr = np.searchsorted(src_l[es], np.arange(V + 1))
    order = np.argsort(W16, kind='stable')
    Wo = W16[order]
    gid = np.zeros(P * COLS2, np.uint8)
    degd = np.zeros(P * COLS2, np.uint8)
    NRI = -(-NRg // 16) * 16
    Lg = np.zeros((NGRP, NRI), np.int64)
    ptr = 0
    rowbase = 0
    for (w16, rowsb, col0) in bins2:
        nb = int(np.searchsorted(Wo, w16, side='right')) - ptr
        nodes = order[ptr:ptr + nb]
        ptr += nb
        if nb:
            k = np.arange(nb)
            g = k % NGRP
            r = rowbase + k // NGRP
            Lg[g, r] = l1pos[nodes]
            cnts = w2[nodes]
            tot = int(cnts.sum())
            colb = col0 + (k // NGRP) * w16
            if tot:
                c0 = np.zeros(nb, np.int64)
                np.cumsum(cnts[:-1], out=c0[1:])
                rep = np.repeat(np.arange(nb), cnts)
                intra = np.arange(tot) - c0[rep]
                ds = dst_sorted[eptr[nodes][rep] + intra]
                cpart = intra % 16
                jj = intra // 16
                pos = (16 * g[rep] + cpart) * COLS2 + colb[rep] + jj
                gid[pos] = batch_u8_g[ds]
                degd[pos] = deg_u8_g[ds]
            cs = cnts % 16
            js = cnts // 16
            pos_s = (16 * g + cs) * COLS2 + colb + js
            gid[pos_s] = batch_u8_g[n0 + nodes]
            degd[pos_s] = deg_u8_g[n0 + nodes]
        rowbase += rowsb
    pidx = np.zeros((128, NRI // 16), np.int16)
    for g in range(NGRP):
        pidx[16 * g:16 * (g + 1), :] = Lg[g].reshape(NRI // 16, 16).T
    return gid, degd, pidx, NRI


# ---------------------------------------------------------------- program

def _bcast_last(ap, k):
    return bass.AP(ap.tensor, ap.offset, list(ap.ap) + [[0, k]])


def _zero_nr(ap, n):
    return bass.AP(ap.tensor, ap.offset, [ap.ap[0], [0, n], ap.ap[1]])


def _build_fused_program(bins, COLS, NR, bins2, COLS2, NRI, dc0, dc1):
    NE = P * NR  # m-table rows
    assert NE <= 2 ** 15, "m table exceeds int16/ap_gather range"
    nc = bacc.Bacc("TRN2", target_bir_lowering=False, debug=False,
                   num_devices=NC)
    x_in = nc.dram_tensor("xp", [P * COLS, 4], f8, kind="ExternalInput")
    a_dv = 448
    a_pi = a_dv + 2 * NR
    MB = -(-(a_pi + 2 * (NRI // 16)) // 4) * 4
    misc_in = nc.dram_tensor("misc", [P, MB], u8, kind="ExternalInput")
    COL2E = COLS2 + (COLS2 & 1)
    gd_in = nc.dram_tensor("gd", [P, COLS2 + COL2E // 2], u8,
                           kind="ExternalInput")
    out = nc.dram_tensor("out", [1, 2 * NUM_GRAPHS], f32,
                         kind="ExternalOutput")

    with tile.TileContext(nc) as tc:
        with (
            tc.tile_pool(name="keep", bufs=1) as kp,
            tc.tile_pool(name="dram", bufs=1, space="DRAM") as dp,
            tc.tile_pool(name="ps", bufs=1, space="PSUM") as ps,
        ):
            gath = kp.tile([P, NRI, 2], f16, tag="gath")

            # ---------------- phase A: conv1 -> m (local nodes, L1 grid)
            with tc.tile_pool(name="ph_a", bufs=1) as ap_:
                xs = ap_.tile([P, COLS, 4], f8, tag="xs")
                nc.sync.dma_start(xs[:], x_in.ap().rearrange("(p c) d -> p c d", p=P))
                dv16 = ap_.tile([P, NR], f16, tag="dv16")
                nc.sync.dma_start(dv16[:],
                                  misc_in.ap()[:, a_dv:a_dv + 2 * NR].bitcast(f16))
                w1 = ap_.tile([P, 64], f32, tag="w1")
                nc.sync.dma_start(w1[:], misc_in.ap()[:, 0:256].bitcast(f32))
                b1 = ap_.tile([P, 16], f32, tag="b1")
                nc.sync.dma_start(b1[:], misc_in.ap()[:, 256:320].bitcast(f32))
                w2 = ap_.tile([P, 32], f32, tag="w2")
                nc.sync.dma_start(w2[:], misc_in.ap()[:, 320:448].bitcast(f32))

                a1 = ap_.tile([P, NR, 4], f32, tag="a1")
                nodecol = 0
                for (w, rows, col0) in bins:
                    sl = xs[:, col0:col0 + rows * w, :]
                    cs = sl.ap[1][0]
                    slT = bass.AP(sl.tensor, sl.offset,
                                  [sl.ap[0], [cs * w, rows], sl.ap[2], [cs, w]])
                    nc.vector.tensor_reduce(
                        out=a1[:, nodecol:nodecol + rows, :], in_=slT,
                        axis=mybir.AxisListType.X, op=ADD)
                    nodecol += rows

                dv = ap_.tile([P, NR], f32, tag="dv")
                nc.vector.tensor_copy(dv[:], dv16[:])
                a1p = ap_.tile([P, NR, 4], f32, tag="a1p")
                nc.vector.tensor_tensor(out=a1p[:], in0=a1[:],
                                        in1=_bcast_last(dv[:], 4), op=MULT)

                h = ap_.tile([P, NR, 16], f32, tag="h")
                tmp = ap_.tile([P, NR, 16], f32, tag="tmp")
                for f in range(4):
                    af = a1p[:, :, f:f + 1]
                    afb = bass.AP(af.tensor, af.offset,
                                  [af.ap[0], af.ap[1], [0, 16]])
                    wfb = _zero_nr(w1[:, f * 16:(f + 1) * 16], NR)
                    if f == 0:
                        nc.vector.tensor_tensor(out=h[:], in0=afb, in1=wfb, op=MULT)
                    else:
                        nc.vector.tensor_tensor(out=tmp[:], in0=afb, in1=wfb, op=MULT)
                        nc.vector.tensor_tensor(out=h[:], in0=h[:], in1=tmp[:], op=ADD)
                nc.vector.tensor_tensor(out=h[:], in0=h[:],
                                        in1=_zero_nr(b1[:], NR), op=ADD)
                nc.vector.tensor_scalar_max(h[:], h[:], 0.0)

                mt = ap_.tile([P, NR, 2], f32, tag="mt")
                for o in range(2):
                    wob = _zero_nr(w2[:, o * 16:(o + 1) * 16], NR)
                    nc.vector.tensor_tensor(out=tmp[:], in0=h[:], in1=wob, op=MULT)
                    nc.vector.tensor_reduce(out=mt[:, :, o], in_=tmp[:],
                                            axis=mybir.AxisListType.X, op=ADD)
                mo = ap_.tile([P, NR, 2], f16, tag="mo")
                nc.vector.tensor_tensor(out=mo[:], in0=mt[:],
                                        in1=_bcast_last(dv[:], 2), op=MULT)
                scratch = dp.tile([P * NR, 2], f16, tag="scratch")
                nc.sync.dma_start(
                    scratch[:].rearrange("(p c) d -> p c d", p=P), mo[:])

            # ---------------- phase B: permute m to L2 block grid
            with tc.tile_pool(name="ph_b", bufs=1) as bp:
                table = bp.tile([P, NE, 2], f16, tag="table")
                sc = scratch[:]
                bcast = bass.AP(sc.tensor, sc.offset, [[0, P], [2, NE], [1, 2]])
                nc.sync.dma_start(table[:], bcast)
                pidx = bp.tile([P, NRI // 16], i16, tag="pidx")
                nc.sync.dma_start(
                    pidx[:],
                    misc_in.ap()[:, a_pi:a_pi + 2 * (NRI // 16)].bitcast(i16))
                nc.gpsimd.ap_gather(gath[:], table[:], pidx[:],
                                    channels=P, num_elems=NE, d=2,
                                    num_idxs=NRI)

            # ---------------- phase C: conv2 + pooling over all graphs
            with tc.tile_pool(name="ph_c", bufs=1) as cpl:
                gt = cpl.tile([P, COLS2], u8, tag="gt")
                nc.sync.dma_start(gt[:], gd_in.ap()[:, :COLS2])
                dp4 = cpl.tile([P, COL2E // 2], u8, tag="dp4")
                nc.sync.dma_start(dp4[:], gd_in.ap()[:, COLS2:])
                ddc = cpl.tile([P, COL2E], u8, tag="ddc")
                dv_ = ddc[:].rearrange("p (c two) -> p c two", two=2)
                nc.vector.tensor_scalar(out=dv_[:, :, 0], in0=dp4[:],
                                        scalar1=15, scalar2=None, op0=BAND)
                nc.vector.tensor_scalar(out=dv_[:, :, 1], in0=dp4[:],
                                        scalar1=4, scalar2=15,
                                        op0=SHR, op1=BAND)
                din = cpl.tile([P, COLS2], f32, tag="din")
                junk = cpl.tile([P, COLS2], f32, tag="junk")
                nc.vector.tensor_scalar(out=junk[:], in0=ddc[:, :COLS2],
                                        scalar1=0.5, scalar2=None, op0=ISGT)
                dcb = cpl.tile([P, 1], f32, tag="dcb")
                nc.vector.memset(dcb[:], float(dc0))
                nc.scalar.activation(din[:], ddc[:, :COLS2],
                                     mybir.ActivationFunctionType.Exp,
                                     bias=dcb[:, 0:1], scale=float(dc1))
                nc.vector.tensor_tensor(out=din[:], in0=din[:], in1=junk[:],
                                        op=MULT)

                v = cpl.tile([P, COLS2, 2], f16, tag="v")
                rowbase = 0
                for (w16, rowsb, col0) in bins2:
                    dst = v[:, col0:col0 + rowsb * w16, :]
                    cs = dst.ap[1][0]
                    dstv = bass.AP(dst.tensor, dst.offset,
                                   [dst.ap[0], [cs * w16, rowsb], [cs, w16],
                                    dst.ap[2]])
                    src = gath[:, rowbase:rowbase + rowsb, :]
                    srcb = bass.AP(src.tensor, src.offset,
                                   [src.ap[0], src.ap[1], [0, w16], src.ap[2]])
                    nc.vector.tensor_copy(dstv, srcb)
                    rowbase += rowsb
                nc.vector.tensor_tensor(out=v[:], in0=v[:],
                                        in1=_bcast_last(din[:], 2), op=MULT)

                po = cpl.tile([P, 2 * NUM_GRAPHS], f32, tag="po")
                for g in range(NUM_GRAPHS):
                    for o in range(2):
                        nc.vector.scalar_tensor_tensor(
                            out=junk[:], in0=gt[:], scalar=float(g),
                            in1=v[:, :, o], op0=ISEQ, op1=MULT,
                            accum_out=po[:, 2 * g + o:2 * g + o + 1])

                ones = cpl.tile([P, 1], f32, tag="ones")
                nc.vector.memset(ones[:], 1.0)
                acc = ps.tile([1, 2 * NUM_GRAPHS], f32, tag="acc")
                nc.tensor.matmul(acc[:], lhsT=ones[:], rhs=po[:],
                                 start=True, stop=True)
                ot = cpl.tile([1, 2 * NUM_GRAPHS], f32, tag="ot")
                nc.vector.tensor_copy(ot[:], acc[:])
                nc.sync.dma_start(out.ap(), ot[:])
    nc.compile()
    return nc


# ---------------------------------------------------------------- pipeline

def _hw_runner(nc, in_maps):
    try:
        return run_bass_kernel_spmd(nc, in_maps, core_ids=list(range(NC)),
                                    trace=TRACE)
    except ModuleNotFoundError:
        return run_bass_kernel_spmd(nc, in_maps, core_ids=list(range(NC)))


def kernel(x, edge_index, batch, W1, b1, W2, b2, Wl, bl, _runner=None):
    runner = _runner or _hw_runner
    x = np.asarray(x, np.float32)
    edge_index = np.asarray(edge_index)
    batch = np.asarray(batch).astype(np.int64)
    N = x.shape[0]
    assert N == N_NODES
    src = edge_index[0].astype(np.int64)
    dst = edge_index[1].astype(np.int64)

    deg_i = np.bincount(dst, minlength=N) + 1
    dinv = 1.0 / np.sqrt(deg_i.astype(np.float32))
    z1f = np.zeros((N + 1, 4), np.float32)
    z1f[:N] = x * dinv[:, None]
    z1b = z1f.astype(F8).view(np.uint8)  # [N+1, 4] fp8 bytes
    DLO = float(deg_i.min())
    DHI = float(max(deg_i.max(), deg_i.min() + 1))
    NLV = 15
    dstep = np.log(DHI / DLO) / (NLV - 1)
    kq = np.clip(np.round(np.log(deg_i / DLO) / dstep), 0, NLV - 1)
    degc_u8_g = (kq + 1).astype(np.uint8)  # 0 reserved for pad
    dc1 = -0.5 * dstep
    dc0 = -0.5 * np.log(DLO) + 0.5 * dstep
    batch_u8_g = batch.astype(np.uint8)

    gb = np.searchsorted(batch, np.arange(NUM_GRAPHS + 1))
    nbounds = [int(gb[GPC * c]) for c in range(NC)] + [N]

    ex = ThreadPoolExecutor(2 * NC)

    def _extract1(c):
        n0, n1 = nbounds[c], nbounds[c + 1]
        mask = (dst >= n0) & (dst < n1)
        return (n0, n1 - n0, src[mask], dst[mask] - n0)

    def _extract2(c):
        n0, n1 = nbounds[c], nbounds[c + 1]
        mask = (src >= n0) & (src < n1)
        sl = src[mask] - n0
        dg = dst[mask]
        w2 = np.bincount(sl, minlength=n1 - n0)
        return sl, dg, (w2 + 1 + 15) // 16

    fut1 = [ex.submit(_extract1, c) for c in range(NC)]
    fut2 = [ex.submit(_extract2, c) for c in range(NC)]
    cores = [f.result() for f in fut1]
    l2ex = [f.result() for f in fut2]
    cores2 = [(sl, dg) for (sl, dg, _) in l2ex]

    maxw = int(deg_i.max())
    hh1 = np.zeros((NC, maxw + 1), np.int64)
    for c, (n0, V, s, d) in enumerate(cores):
        hh1[c] = np.bincount(deg_i[n0:n0 + V], minlength=maxw + 1)
    bins, COLS, NR = _dp_bins(hh1, maxw, P, P, 2048)

    w16max = max(int(W16.max()) for (_, _, W16) in l2ex)
    hh2 = np.zeros((NC * NGRP, w16max + 1), np.int64)
    for c in range(NC):
        # deal within sorted order -> per-group counts differ by <=1;
        # conservatively use ceil(count/NGRP) per width as per-group count
        hc = np.bincount(l2ex[c][2], minlength=w16max + 1)
        hh2[c * NGRP] = -(-hc // NGRP)
    bins2, COLS2, NRg = _dp_bins(hh2, w16max, 1, 16, 256)

    def _stage_core(c):
        n0, n1 = nbounds[c], nbounds[c + 1]
        V = n1 - n0
        st1 = _stage_l1(n0, V, cores[c][2], cores[c][3], deg_i[n0:n0 + V],
                        bins, COLS, NR, z1b, dinv)
        node_map = st1[2]
        l1pos = np.zeros(V, np.int64)
        nm = node_map.reshape(-1)
        valid = nm >= 0
        l1pos[nm[valid]] = np.arange(P * NR)[valid]
        sl, dg = cores2[c]
        st2 = _stage_l2_blk(n0, V, sl, dg, bins2, COLS2, NRg,
                            degc_u8_g, batch_u8_g, l1pos)
        return st1, st2

    staged = list(ex.map(_stage_core, range(NC)))
    ex.shutdown(wait=False)
    l1_stage = [s[0] for s in staged]
    l2_stage = [s[1] for s in staged]
    NRI = l2_stage[0][3]

    W1 = np.asarray(W1, np.float32)
    W2p = np.asarray(W2, np.float32) @ np.asarray(Wl, np.float32)
    W1r = np.broadcast_to(W1.reshape(1, 64), (P, 64)).copy()
    b1r = np.broadcast_to(np.asarray(b1, np.float32).reshape(1, 16),
                          (P, 16)).copy()
    W2pr = np.broadcast_to(W2p.T.reshape(1, 32), (P, 32)).copy()

    key = ("fused", tuple(bins), tuple(bins2),
           round(float(dc0), 9), round(float(dc1), 9))
    if key not in _PROG_CACHE:
        _PROG_CACHE[key] = _build_fused_program(bins, COLS, NR,
                                                bins2, COLS2, NRI, dc0, dc1)
    nc = _PROG_CACHE[key]
    a_dv = 448
    a_pi = a_dv + 2 * NR
    MB = -(-(a_pi + 2 * (NRI // 16)) // 4) * 4
    in_maps = []
    for c in range(NC):
        x_slot, dinvn, _ = l1_stage[c]
        gid, degd, pidx, _ = l2_stage[c]
        misc = np.zeros((P, MB), np.uint8)
        misc[:, 0:256] = W1r.view(np.uint8)
        misc[:, 256:320] = b1r.view(np.uint8)
        misc[:, 320:448] = W2pr.view(np.uint8)
        misc[:, a_dv:a_dv + 2 * NR] = dinvn.view(np.uint8)
        misc[:, a_pi:a_pi + 2 * (NRI // 16)] = pidx.view(np.uint8)
        COL2E = COLS2 + (COLS2 & 1)
        dge = np.zeros((P, COL2E), np.uint8)
        dge[:, :COLS2] = degd.reshape(P, COLS2)
        dp4 = (dge[:, 0::2] | (dge[:, 1::2] << 4)).astype(np.uint8)
        gd = np.concatenate([gid.reshape(P, COLS2), dp4], axis=1)
        in_maps.append({"xp": x_slot, "misc": misc, "gd": gd})
    t0 = _time.time()
    res = runner(nc, in_maps)
    LAST_WALL_S.append(_time.time() - t0)
    LAST_RESULTS.append(res)
    LAST_EXEC_NS.append(res.exec_time_ns)

    bias = (np.asarray(b2, np.float32) @ np.asarray(Wl, np.float32)
            + np.asarray(bl, np.float32))
    sizes = np.diff(gb).astype(np.float32)
    acc = np.zeros((NUM_GRAPHS, 2), np.float32)
    for c in range(NC):
        acc += np.asarray(res.results[c]["out"]).reshape(NUM_GRAPHS, 2)
    out = acc / np.maximum(sizes, 1.0)[:, None] + bias[None, :]
    out[sizes == 0] = 0.0
    return out
